# revision 5
# baseline (speedup 1.0000x reference)
"""Multi-head attention (B=4, N=2048, D=1024, H=16) on 8 TRN2 NeuronCores.

Sharding: DP=4 over batch x TP=2 over heads (megatron style).
  core c = 2*batch + j   (j in {0,1} = head-group half)
  - inputs:  x[batch] transposed -> xT [D, N]  (bf16)
  - W_qkv column-sharded: wq/wk/wv = W_qkv[:, {q,k,v} block, heads j*8:(j+1)*8]
  - W_proj row-sharded:   wp = W_proj[j*512:(j+1)*512, :]
  - per-core partial out [N, D]; host sums the TP pair (+ bias via per-core
    bias input that is b_proj on j==0 and zeros on j==1).

Per-core math (bf16 matmuls, fp32 PSUM accumulation):
  qT/kT  [feat, tok] = W^T @ xT          (feature-major, head pairs stacked)
  v      [tok, feat]                     (token-major), augmented with a
                                         ones-column per head (softmax sums)
  per head h, query-tile qh:
    S^T chunk [128 keys, QT] = kT_chunk.T @ qT     (K = hd = 64)
    A^T = exp(scale * S^T)  (ScalarE, PSUM->SBUF, bf16)
    outT_aug [65, QT] += v_aug_chunk.T @ A^T       (row 64 = softmax sums)
    normalize: bcast sums via K=1 matmul, reciprocal_approx_fast, multiply
    -> att_outT [feat, tok] (feature-major == proj lhsT layout)
  proj: out[tok_block] = att_outT_chunk.T @ wp + bias
"""

import numpy as np

B, N, D, H = 4, 2048, 1024, 16
HD = 64
NCORES = 8
TP = 2
HLOC = H // TP          # 8 heads per core
FDIM = HLOC * HD        # 512

_PROG_CACHE = {}


def _build_program(tok, d, h_loc, hd, debug=False):
    """Build the single-core Bass/Tile program (same program runs SPMD on all cores)."""
    import concourse.tile as tile
    from concourse import bacc, mybir

    f32 = mybir.dt.float32
    bf16 = mybir.dt.bfloat16
    Exp = mybir.ActivationFunctionType.Exp

    P = 128
    DC = d // P                 # contraction chunks for QKV (8)
    NP = h_loc // 2             # head pairs (4)
    TB = tok // P               # token blocks (16)
    KC = tok // P               # key chunks (16)
    fdim = h_loc * hd           # local feature dim (512)
    FC = fdim // P              # proj contraction chunks (4)
    QT = min(tok, 1024)         # query tile
    QH = tok // QT              # query-halves (2)
    MMN = 512                   # max matmul free dim per instruction
    scale = float(hd) ** -0.5

    nc = bacc.Bacc("TRN2", target_bir_lowering=False, debug=debug)

    xT = nc.dram_tensor("xT", [d, tok], bf16, kind="ExternalInput")
    wq = nc.dram_tensor("wq", [d, fdim], bf16, kind="ExternalInput")
    wk = nc.dram_tensor("wk", [d, fdim], bf16, kind="ExternalInput")
    wv = nc.dram_tensor("wv", [d, fdim], bf16, kind="ExternalInput")
    wp = nc.dram_tensor("wp", [fdim, d], bf16, kind="ExternalInput")
    bias = nc.dram_tensor("bias", [P, d], f32, kind="ExternalInput")
    out = nc.dram_tensor("out", [tok, d], f32, kind="ExternalOutput")

    with tile.TileContext(nc) as tc:
        with (
            tc.tile_pool(name="sing", bufs=1) as sing,
            tc.tile_pool(name="psA", bufs=2, space="PSUM") as psA,
            tc.tile_pool(name="psO", bufs=2, space="PSUM") as psO,
            tc.tile_pool(name="work", bufs=3) as work,
            tc.tile_pool(name="outp", bufs=3) as outp,
        ):
            # ---- resident loads -------------------------------------------------
            xT_sb = sing.tile([P, DC, tok], bf16)
            nc.sync.dma_start(out=xT_sb, in_=xT[:, :].rearrange("(c p) n -> p c n", p=P))
            wq_sb = sing.tile([P, DC, fdim], bf16)
            nc.sync.dma_start(out=wq_sb, in_=wq[:, :].rearrange("(c p) m -> p c m", p=P))
            wk_sb = sing.tile([P, DC, fdim], bf16)
            nc.sync.dma_start(out=wk_sb, in_=wk[:, :].rearrange("(c p) m -> p c m", p=P))
            wv_sb = sing.tile([P, DC, fdim], bf16)
            nc.sync.dma_start(out=wv_sb, in_=wv[:, :].rearrange("(c p) m -> p c m", p=P))
            wp_sb = sing.tile([P, FC, d], bf16)
            nc.sync.dma_start(out=wp_sb, in_=wp[:, :].rearrange("(c p) m -> p c m", p=P))
            bias_sb = sing.tile([P, d], f32)
            nc.sync.dma_start(out=bias_sb, in_=bias[:, :])

            ones_sb = sing.tile([1, hd], f32)
            nc.vector.memset(ones_sb, 1.0)

            qT_sb = sing.tile([P, NP, tok], bf16)
            kT_sb = sing.tile([P, NP, tok], bf16)
            vaug_sb = sing.tile([P, KC, h_loc, hd + 1], bf16)
            nc.vector.memset(vaug_sb, 1.0)  # ones column survives the v copies
            aoT_sb = sing.tile([P, NP, tok], bf16)

            # ---- stage 1: qT / kT (feature-major, head-pair stacked) -----------
            for w_sb, dst in ((wq_sb, qT_sb), (wk_sb, kT_sb)):
                for p in range(NP):
                    for n0 in range(0, tok, QT):
                        ps = psA.tile([P, QT], f32, tag="ps")
                        for c in range(DC):
                            for m0 in range(0, QT, MMN):
                                ml = min(MMN, QT - m0)
                                nc.tensor.matmul(
                                    ps[:, m0:m0 + ml],
                                    w_sb[:, c, p * P:(p + 1) * P],
                                    xT_sb[:, c, n0 + m0:n0 + m0 + ml],
                                    start=(c == 0),
                                    stop=(c == DC - 1),
                                )
                        nc.vector.tensor_copy(dst[:, p, n0:n0 + QT], ps)

            # ---- stage 1b: v (token-major) into vaug ---------------------------
            for tb in range(TB):
                ps = psA.tile([P, QT], f32, tag="ps")
                for c in range(DC):
                    for m0 in range(0, fdim, MMN):
                        ml = min(MMN, fdim - m0)
                        nc.tensor.matmul(
                            ps[:, m0:m0 + ml],
                            xT_sb[:, c, tb * P:(tb + 1) * P],
                            wv_sb[:, c, m0:m0 + ml],
                            start=(c == 0),
                            stop=(c == DC - 1),
                        )
                nc.vector.tensor_copy(
                    vaug_sb[:, tb, :, 0:hd],
                    ps[:, 0:fdim].rearrange("p (h e) -> p h e", h=h_loc),
                )

            # ---- stage 2: attention per head -----------------------------------
            for h in range(h_loc):
                p, e = divmod(h, 2)
                for qh in range(QH):
                    q0 = qh * QT
                    po = psO.tile([hd + 1, QT], f32, tag="po")
                    for kc in range(KC):
                        ps = psA.tile([P, QT], f32, tag="ps")
                        for m0 in range(0, QT, MMN):
                            ml = min(MMN, QT - m0)
                            nc.tensor.matmul(
                                ps[:, m0:m0 + ml],
                                kT_sb[e * hd:(e + 1) * hd, p, kc * P:(kc + 1) * P],
                                qT_sb[e * hd:(e + 1) * hd, p, q0 + m0:q0 + m0 + ml],
                                start=True,
                                stop=True,
                            )
                        at = work.tile([P, QT], bf16, tag="at")
                        nc.scalar.activation(at, ps, Exp, scale=scale)
                        for m0 in range(0, QT, MMN):
                            ml = min(MMN, QT - m0)
                            nc.tensor.matmul(
                                po[:, m0:m0 + ml],
                                vaug_sb[:, kc, h, :],
                                at[:, m0:m0 + ml],
                                start=(kc == 0),
                                stop=(kc == KC - 1),
                            )
                    # normalize: recip(sums) broadcast over the hd partitions
                    sums = work.tile([1, QT], f32, tag="sums")
                    nc.vector.tensor_copy(sums, po[hd:hd + 1, :])
                    pb = psA.tile([P, QT], f32, tag="ps")
                    for m0 in range(0, QT, MMN):
                        ml = min(MMN, QT - m0)
                        nc.tensor.matmul(
                            pb[0:hd, m0:m0 + ml],
                            ones_sb,
                            sums[:, m0:m0 + ml],
                            start=True,
                            stop=True,
                        )
                    rec = work.tile([hd, QT], f32, tag="rec")
                    nc.vector.reciprocal_approx_fast(rec, pb[0:hd, :])
                    nc.vector.tensor_mul(
                        aoT_sb[e * hd:(e + 1) * hd, p, q0:q0 + QT],
                        po[0:hd, :],
                        rec,
                    )

            # ---- stage 3: projection + bias ------------------------------------
            for tb in range(TB):
                pp = psA.tile([P, d], f32, tag="ps")
                for fc in range(FC):
                    for m0 in range(0, d, MMN):
                        ml = min(MMN, d - m0)
                        nc.tensor.matmul(
                            pp[:, m0:m0 + ml],
                            aoT_sb[:, fc, tb * P:(tb + 1) * P],
                            wp_sb[:, fc, m0:m0 + ml],
                            start=(fc == 0),
                            stop=(fc == FC - 1),
                        )
                ot = outp.tile([P, d], f32, tag="ot")
                nc.vector.tensor_add(ot, pp, bias_sb)
                nc.sync.dma_start(out=out[tb * P:(tb + 1) * P, :], in_=ot)

    nc.compile()
    return nc


def get_program(tok=N, d=D, h_loc=HLOC, hd=HD, debug=False):
    key = (tok, d, h_loc, hd, debug)
    if key not in _PROG_CACHE:
        _PROG_CACHE[key] = _build_program(tok, d, h_loc, hd, debug=debug)
    return _PROG_CACHE[key]


def make_in_maps(inputs_f32, W_qkv, W_proj, b_proj):
    """Shard full inputs into the 8 per-core input dicts."""
    import ml_dtypes

    bf16 = ml_dtypes.bfloat16
    in_maps = []
    for core in range(NCORES):
        b_idx, j = divmod(core, TP)
        f0, f1 = j * FDIM, (j + 1) * FDIM
        xT = np.ascontiguousarray(inputs_f32[b_idx].T).astype(bf16)
        wq_s = np.ascontiguousarray(W_qkv[:, f0:f1]).astype(bf16)
        wk_s = np.ascontiguousarray(W_qkv[:, D + f0:D + f1]).astype(bf16)
        wv_s = np.ascontiguousarray(W_qkv[:, 2 * D + f0:2 * D + f1]).astype(bf16)
        wp_s = np.ascontiguousarray(W_proj[f0:f1, :]).astype(bf16)
        if j == 0:
            bias_rep = np.broadcast_to(b_proj.astype(np.float32), (128, D)).copy()
        else:
            bias_rep = np.zeros((128, D), np.float32)
        in_maps.append(
            {"xT": xT, "wq": wq_s, "wk": wk_s, "wv": wv_s, "wp": wp_s,
             "bias": bias_rep}
        )
    return in_maps


def kernel(inputs, W_qkv, W_proj, b_proj):
    from concourse.bass_utils import run_bass_kernel_spmd

    inputs = np.asarray(inputs, dtype=np.float32)
    W_qkv = np.asarray(W_qkv, dtype=np.float32)
    W_proj = np.asarray(W_proj, dtype=np.float32)
    b_proj = np.asarray(b_proj, dtype=np.float32)

    nc = get_program()
    in_maps = make_in_maps(inputs, W_qkv, W_proj, b_proj)
    res = run_bass_kernel_spmd(nc, in_maps, core_ids=list(range(NCORES)))
    outs = [r["out"].astype(np.float32) for r in res.results]
    full = np.stack([outs[TP * b] + outs[TP * b + 1] for b in range(B)], axis=0)
    return full


# revision 10
# speedup vs baseline: 10.2414x; 10.2414x over previous
"""Multi-head attention (B=4, N=2048, D=1024, H=16) on 8 TRN2 NeuronCores.

Sharding: DP=4 over batch x TP=2 over heads (megatron style).
  core c = 2*batch + j   (j in {0,1} = head-group half)
  - inputs:  x[batch] transposed -> xT [D, N]  (bf16)
  - W_qkv column-sharded: wq/wk/wv = W_qkv[:, {q,k,v} block, heads j*8:(j+1)*8]
  - W_proj row-sharded:   wp = W_proj[j*512:(j+1)*512, :]
  - per-core partial out [N, D]; host sums the TP pair (+ bias via per-core
    bias input that is b_proj on j==0 and zeros on j==1).

Per-core math (bf16 matmuls, fp32 PSUM accumulation):
  qT/kT  [feat, tok] = W^T @ xT          (feature-major, head pairs stacked)
  v      [tok, feat]                     (token-major), augmented with a
                                         ones-column per head (softmax sums)
  per head h, query-tile qh:
    S^T chunk [128 keys, QT] = kT_chunk.T @ qT     (K = hd = 64)
    A^T = exp(scale * S^T)  (ScalarE, PSUM->SBUF, bf16)
    outT_aug [65, QT] += v_aug_chunk.T @ A^T       (row 64 = softmax sums)
    normalize: bcast sums via K=1 matmul, reciprocal_approx_fast, multiply
    -> att_outT [feat, tok] (feature-major == proj lhsT layout)
  proj: out[tok_block] = att_outT_chunk.T @ wp + bias
"""

import numpy as np

B, N, D, H = 4, 2048, 1024, 16
HD = 64
NCORES = 8
TP = 2
HLOC = H // TP          # 8 heads per core
FDIM = HLOC * HD        # 512

_PROG_CACHE = {}


def _build_program(tok, d, h_loc, hd, debug=False, repeat=1):
    """Build the single-core Bass/Tile program (same program runs SPMD on all cores)."""
    import concourse.tile as tile
    from concourse import bacc, mybir

    f32 = mybir.dt.float32
    bf16 = mybir.dt.bfloat16
    Exp = mybir.ActivationFunctionType.Exp

    P = 128
    DC = d // P                 # contraction chunks for QKV (8)
    NP = h_loc // 2             # head pairs (4)
    TB = tok // P               # token blocks (16)
    KC = tok // P               # key chunks (16)
    fdim = h_loc * hd           # local feature dim (512)
    FC = fdim // P              # proj contraction chunks (4)
    QT = min(tok, 1024)         # query tile
    QH = tok // QT              # query-halves (2)
    MMN = 512                   # max matmul free dim per instruction
    scale = float(hd) ** -0.5

    nc = bacc.Bacc("TRN2", target_bir_lowering=False, debug=debug)

    xT = nc.dram_tensor("xT", [d, tok], bf16, kind="ExternalInput")
    wq = nc.dram_tensor("wq", [d, fdim], bf16, kind="ExternalInput")
    wk = nc.dram_tensor("wk", [d, fdim], bf16, kind="ExternalInput")
    wv = nc.dram_tensor("wv", [d, fdim], bf16, kind="ExternalInput")
    wp = nc.dram_tensor("wp", [fdim, d], bf16, kind="ExternalInput")
    bias = nc.dram_tensor("bias", [P, d], f32, kind="ExternalInput")
    out = nc.dram_tensor("out", [tok, d], f32, kind="ExternalOutput")

    with tile.TileContext(nc) as tc:
        with (
            tc.tile_pool(name="sing", bufs=1) as sing,
            tc.tile_pool(name="psA", bufs=2, space="PSUM") as psA,
            tc.tile_pool(name="psO", bufs=2, space="PSUM") as psO,
            tc.tile_pool(name="work", bufs=3) as work,
            tc.tile_pool(name="outp", bufs=3) as outp,
        ):
          for _rep in range(repeat):
            # ---- resident loads (xT chunked so compute starts ASAP) -----------
            xT_sb = sing.tile([P, DC, tok], bf16)
            for c in range(DC):
                nc.sync.dma_start(
                    out=xT_sb[:, c, :],
                    in_=xT[c * P:(c + 1) * P, :])
            wv_sb = sing.tile([P, DC, fdim], bf16)
            nc.sync.dma_start(out=wv_sb, in_=wv[:, :].rearrange("(c p) m -> p c m", p=P))
            wq_sb = sing.tile([P, DC, fdim], bf16)
            nc.sync.dma_start(out=wq_sb, in_=wq[:, :].rearrange("(c p) m -> p c m", p=P))
            wk_sb = sing.tile([P, DC, fdim], bf16)
            nc.sync.dma_start(out=wk_sb, in_=wk[:, :].rearrange("(c p) m -> p c m", p=P))
            wp_sb = sing.tile([P, FC, d], bf16)
            nc.sync.dma_start(out=wp_sb, in_=wp[:, :].rearrange("(c p) m -> p c m", p=P))
            bias_sb = sing.tile([P, d], f32)
            nc.sync.dma_start(out=bias_sb, in_=bias[:, :])

            qT_sb = sing.tile([P, NP, tok], bf16)
            kT_sb = sing.tile([P, NP, tok], bf16)
            vaug_sb = sing.tile([P, KC, h_loc, hd + 1], bf16)
            nc.vector.memset(vaug_sb, 1.0)  # ones column survives the v copies
            aoT_sb = sing.tile([P, NP, tok], bf16)

            # ---- v (token-major) into vaug -------------------------------------
            for tb in range(TB):
                ps = psA.tile([P, QT], f32, tag="ps")
                for c in range(DC):
                    for m0 in range(0, fdim, MMN):
                        ml = min(MMN, fdim - m0)
                        nc.tensor.matmul(
                            ps[:, m0:m0 + ml],
                            xT_sb[:, c, tb * P:(tb + 1) * P],
                            wv_sb[:, c, m0:m0 + ml],
                            start=(c == 0),
                            stop=(c == DC - 1),
                        )
                nc.vector.tensor_copy(
                    vaug_sb[:, tb, :, 0:hd],
                    ps[:, 0:fdim].rearrange("p (h e) -> p h e", h=h_loc),
                )

            # ---- per head pair: q/k projections then attention -----------------
            for p in range(NP):
                for w_sb, dst in ((wq_sb, qT_sb), (wk_sb, kT_sb)):
                    for n0 in range(0, tok, QT):
                        ps = psA.tile([P, QT], f32, tag="ps")
                        for c in range(DC):
                            for m0 in range(0, QT, MMN):
                                ml = min(MMN, QT - m0)
                                nc.tensor.matmul(
                                    ps[:, m0:m0 + ml],
                                    w_sb[:, c, p * P:(p + 1) * P],
                                    xT_sb[:, c, n0 + m0:n0 + m0 + ml],
                                    start=(c == 0),
                                    stop=(c == DC - 1),
                                )
                        nc.vector.tensor_copy(dst[:, p, n0:n0 + QT], ps)

                for qh in range(QH):
                    q0 = qh * QT
                    # both parities of the pair interleaved: the parity-1
                    # S^T matmuls use base_partition 64 (row groups 2-3), so
                    # the PE can run them concurrently with parity-0.
                    pos = [psO.tile([hd + 1, QT], f32, tag="po", name=f"po{_e}")
                           for _e in range(2)]
                    for kc in range(KC):
                        ats = []
                        for e in range(2):
                            ps = psA.tile([P, QT], f32, tag="ps")
                            for m0 in range(0, QT, MMN):
                                ml = min(MMN, QT - m0)
                                nc.tensor.matmul(
                                    ps[:, m0:m0 + ml],
                                    kT_sb[e * hd:(e + 1) * hd, p, kc * P:(kc + 1) * P],
                                    qT_sb[e * hd:(e + 1) * hd, p, q0 + m0:q0 + m0 + ml],
                                    start=True,
                                    stop=True,
                                )
                            at = work.tile([P, QT], bf16, tag="at")
                            nc.scalar.activation(at, ps, Exp, scale=scale)
                            ats.append(at)
                        for e in range(2):
                            for m0 in range(0, QT, MMN):
                                ml = min(MMN, QT - m0)
                                nc.tensor.matmul(
                                    pos[e][:, m0:m0 + ml],
                                    vaug_sb[:, kc, 2 * p + e, :],
                                    ats[e][:, m0:m0 + ml],
                                    start=(kc == 0),
                                    stop=(kc == KC - 1),
                                )
                    for e in range(2):
                        po = pos[e]
                        # sums row psum->sbuf, then stride-0 DMA broadcast
                        # to hd partitions (DMA engines are otherwise idle)
                        sums = work.tile([1, QT], f32, tag="sums")
                        nc.vector.tensor_copy(sums, po[hd:hd + 1, :])
                        rec_src = work.tile([hd, QT], f32, tag="rsrc")
                        nc.sync.dma_start(
                            out=rec_src,
                            in_=sums.partition_broadcast(hd))
                        rec = work.tile([hd, QT], f32, tag="rec")
                        nc.vector.reciprocal_approx_fast(rec, rec_src)
                        nc.vector.tensor_mul(
                            aoT_sb[e * hd:(e + 1) * hd, p, q0:q0 + QT],
                            po[0:hd, :],
                            rec,
                        )

            # ---- projection + bias ---------------------------------------------
            for tb in range(TB):
                pp = psA.tile([P, d], f32, tag="ps")
                for fc in range(FC):
                    for m0 in range(0, d, MMN):
                        ml = min(MMN, d - m0)
                        nc.tensor.matmul(
                            pp[:, m0:m0 + ml],
                            aoT_sb[:, fc, tb * P:(tb + 1) * P],
                            wp_sb[:, fc, m0:m0 + ml],
                            start=(fc == 0),
                            stop=(fc == FC - 1),
                        )
                ot = outp.tile([P, d], f32, tag="ot")
                nc.vector.tensor_add(ot, pp, bias_sb)
                nc.sync.dma_start(out=out[tb * P:(tb + 1) * P, :], in_=ot)

    nc.compile()
    return nc


def get_program(tok=N, d=D, h_loc=HLOC, hd=HD, debug=False, repeat=1):
    key = (tok, d, h_loc, hd, debug, repeat)
    if key not in _PROG_CACHE:
        _PROG_CACHE[key] = _build_program(tok, d, h_loc, hd, debug=debug,
                                          repeat=repeat)
    return _PROG_CACHE[key]


def make_in_maps(inputs_f32, W_qkv, W_proj, b_proj):
    """Shard full inputs into the 8 per-core input dicts."""
    import ml_dtypes

    bf16 = ml_dtypes.bfloat16
    in_maps = []
    for core in range(NCORES):
        b_idx, j = divmod(core, TP)
        f0, f1 = j * FDIM, (j + 1) * FDIM
        xT = np.ascontiguousarray(inputs_f32[b_idx].T).astype(bf16)
        wq_s = np.ascontiguousarray(W_qkv[:, f0:f1]).astype(bf16)
        wk_s = np.ascontiguousarray(W_qkv[:, D + f0:D + f1]).astype(bf16)
        wv_s = np.ascontiguousarray(W_qkv[:, 2 * D + f0:2 * D + f1]).astype(bf16)
        wp_s = np.ascontiguousarray(W_proj[f0:f1, :]).astype(bf16)
        if j == 0:
            bias_rep = np.broadcast_to(b_proj.astype(np.float32), (128, D)).copy()
        else:
            bias_rep = np.zeros((128, D), np.float32)
        in_maps.append(
            {"xT": xT, "wq": wq_s, "wk": wk_s, "wv": wv_s, "wp": wp_s,
             "bias": bias_rep}
        )
    return in_maps


def kernel(inputs, W_qkv, W_proj, b_proj):
    from concourse.bass_utils import run_bass_kernel_spmd

    inputs = np.asarray(inputs, dtype=np.float32)
    W_qkv = np.asarray(W_qkv, dtype=np.float32)
    W_proj = np.asarray(W_proj, dtype=np.float32)
    b_proj = np.asarray(b_proj, dtype=np.float32)

    nc = get_program()
    in_maps = make_in_maps(inputs, W_qkv, W_proj, b_proj)
    res = run_bass_kernel_spmd(nc, in_maps, core_ids=list(range(NCORES)))
    outs = [r["out"].astype(np.float32) for r in res.results]
    full = np.stack([outs[TP * b] + outs[TP * b + 1] for b in range(B)], axis=0)
    return full


# revision 15
# speedup vs baseline: 21.9596x; 2.1442x over previous
"""Multi-head attention (B=4, N=2048, D=1024, H=16) on 8 TRN2 NeuronCores.

Sharding: DP=4 over batch x TP=2 over heads (megatron style).
  core c = 2*batch + j   (j in {0,1} = head-group half)
  - inputs:  x[batch] transposed -> xT [D, N]  (bf16)
  - W_qkv column-sharded: wq/wk/wv = W_qkv[:, {q,k,v} block, heads j*8:(j+1)*8]
  - W_proj row-sharded:   wp = W_proj[j*512:(j+1)*512, :]
  - per-core partial out [N, D]; host sums the TP pair (+ bias via per-core
    bias input that is b_proj on j==0 and zeros on j==1).

Per-core math (bf16 matmuls, fp32 PSUM accumulation):
  qT/kT  [feat, tok] = W^T @ xT          (feature-major, head pairs stacked)
  v      [tok, feat]                     (token-major), augmented with a
                                         ones-column per head (softmax sums)
  per head h, query-tile qh:
    S^T chunk [128 keys, QT] = kT_chunk.T @ qT     (K = hd = 64)
    A^T = exp(scale * S^T)  (ScalarE, PSUM->SBUF, bf16)
    outT_aug [65, QT] += v_aug_chunk.T @ A^T       (row 64 = softmax sums)
    normalize: bcast sums via K=1 matmul, reciprocal_approx_fast, multiply
    -> att_outT [feat, tok] (feature-major == proj lhsT layout)
  proj: out[tok_block] = att_outT_chunk.T @ wp + bias
"""

import numpy as np

B, N, D, H = 4, 2048, 1024, 16
HD = 64
NCORES = 8
TP = 2
HLOC = H // TP          # 8 heads per core
FDIM = HLOC * HD        # 512

_PROG_CACHE = {}


def _build_program(tok, d, h_loc, hd, debug=False, repeat=1):
    """Build the single-core Bass/Tile program (same program runs SPMD on all cores)."""
    import concourse.tile as tile
    from concourse import bacc, mybir

    f32 = mybir.dt.float32
    bf16 = mybir.dt.bfloat16
    Exp = mybir.ActivationFunctionType.Exp

    P = 128
    DC = d // P                 # contraction chunks for QKV (8)
    NP = h_loc // 2             # head pairs (4)
    TB = tok // P               # token blocks (16)
    KC = tok // P               # key chunks (16)
    fdim = h_loc * hd           # local feature dim (512)
    FC = fdim // P              # proj contraction chunks (4)
    QT = min(tok, 1024)         # query tile
    QH = tok // QT              # query-halves (2)
    MMN = 512                   # max matmul free dim per instruction
    scale = float(hd) ** -0.5

    nc = bacc.Bacc("TRN2", target_bir_lowering=False, debug=debug)

    xT = nc.dram_tensor("xT", [d, tok], bf16, kind="ExternalInput")
    wq = nc.dram_tensor("wq", [d, fdim], bf16, kind="ExternalInput")
    wk = nc.dram_tensor("wk", [d, fdim], bf16, kind="ExternalInput")
    wv = nc.dram_tensor("wv", [d, fdim], bf16, kind="ExternalInput")
    wp = nc.dram_tensor("wp", [fdim, d], bf16, kind="ExternalInput")
    bias = nc.dram_tensor("bias", [P, d], f32, kind="ExternalInput")
    out = nc.dram_tensor("out", [tok, d], f32, kind="ExternalOutput")

    with tile.TileContext(nc) as tc:
        with (
            tc.tile_pool(name="sing", bufs=1) as sing,
            tc.tile_pool(name="psA", bufs=2, space="PSUM") as psA,
            tc.tile_pool(name="psO", bufs=2, space="PSUM") as psO,
            tc.tile_pool(name="work", bufs=3) as work,
            tc.tile_pool(name="outp", bufs=3) as outp,
            tc.tile_pool(name="dscr", bufs=4, space="DRAM") as dscr,
        ):
          for _rep in range(repeat):
            # ---- resident loads (xT chunked so compute starts ASAP) -----------
            wv_sb = sing.tile([P, DC, fdim], bf16)
            nc.gpsimd.dma_start(out=wv_sb, in_=wv[:, :].rearrange("(c p) m -> p c m", p=P))
            xT_sb = sing.tile([P, DC, tok], bf16)
            for c in range(DC):
                nc.sync.dma_start(
                    out=xT_sb[:, c, :],
                    in_=xT[c * P:(c + 1) * P, :])
            wq_sb = sing.tile([P, DC, fdim], bf16)
            nc.gpsimd.dma_start(out=wq_sb, in_=wq[:, :].rearrange("(c p) m -> p c m", p=P))
            wk_sb = sing.tile([P, DC, fdim], bf16)
            nc.gpsimd.dma_start(out=wk_sb, in_=wk[:, :].rearrange("(c p) m -> p c m", p=P))
            wp_sb = sing.tile([P, FC, d], bf16)
            nc.gpsimd.dma_start(out=wp_sb, in_=wp[:, :].rearrange("(c p) m -> p c m", p=P))
            bias_sb = sing.tile([P, d], f32)
            nc.gpsimd.dma_start(out=bias_sb, in_=bias[:, :])

            qT_sb = sing.tile([P, NP, tok], bf16)
            kT_sb = sing.tile([P, NP, tok], bf16)
            vaug_sb = sing.tile([P, KC, h_loc, hd + 1], bf16)
            nc.vector.memset(vaug_sb, 1.0)  # ones column survives the v copies
            aoT_sb = sing.tile([P, NP, tok], bf16)

            # ---- v (token-major) into vaug -------------------------------------
            for tb in range(TB):
                ps = psA.tile([P, QT], f32, tag="ps")
                for c in range(DC):
                    for m0 in range(0, fdim, MMN):
                        ml = min(MMN, fdim - m0)
                        nc.tensor.matmul(
                            ps[:, m0:m0 + ml],
                            xT_sb[:, c, tb * P:(tb + 1) * P],
                            wv_sb[:, c, m0:m0 + ml],
                            start=(c == 0),
                            stop=(c == DC - 1),
                        )
                nc.vector.tensor_copy(
                    vaug_sb[:, tb, :, 0:hd],
                    ps[:, 0:fdim].rearrange("p (h e) -> p h e", h=h_loc),
                )

            # ---- q/k projections (all pairs up front) --------------------------
            for p in range(NP):
                for w_sb, dst in ((wq_sb, qT_sb), (wk_sb, kT_sb)):
                    for n0 in range(0, tok, QT):
                        ps = psA.tile([P, QT], f32, tag="ps")
                        for c in range(DC):
                            for m0 in range(0, QT, MMN):
                                ml = min(MMN, QT - m0)
                                nc.tensor.matmul(
                                    ps[:, m0:m0 + ml],
                                    w_sb[:, c, p * P:(p + 1) * P],
                                    xT_sb[:, c, n0 + m0:n0 + m0 + ml],
                                    start=(c == 0),
                                    stop=(c == DC - 1),
                                )
                        nc.vector.tensor_copy(dst[:, p, n0:n0 + QT], ps)

            # ---- attention (qh outer so proj can interleave) -------------------
            def proj_block(tb):
                pp = psA.tile([P, d], f32, tag="ps", name="pp")
                for fc in range(FC):
                    for m0 in range(0, d, MMN):
                        ml = min(MMN, d - m0)
                        nc.tensor.matmul(
                            pp[:, m0:m0 + ml],
                            aoT_sb[:, fc, tb * P:(tb + 1) * P],
                            wp_sb[:, fc, m0:m0 + ml],
                            start=(fc == 0),
                            stop=(fc == FC - 1),
                        )
                ot = outp.tile([P, d], f32, tag="ot", name="ot")
                nc.vector.tensor_add(ot, pp, bias_sb)
                nc.sync.dma_start(out=out[tb * P:(tb + 1) * P, :], in_=ot)

            for qh in range(QH):
                q0 = qh * QT
                for p in range(NP):
                    pos = [psO.tile([hd + 1, QT], f32, tag="po", name=f"po{_e}")
                           for _e in range(2)]
                    for kc in range(KC):
                        ats = []
                        for e in range(2):
                            ps = psA.tile([P, QT], f32, tag="ps")
                            for m0 in range(0, QT, MMN):
                                ml = min(MMN, QT - m0)
                                nc.tensor.matmul(
                                    ps[:, m0:m0 + ml],
                                    kT_sb[e * hd:(e + 1) * hd, p, kc * P:(kc + 1) * P],
                                    qT_sb[e * hd:(e + 1) * hd, p, q0 + m0:q0 + m0 + ml],
                                    start=True,
                                    stop=True,
                                )
                            at = work.tile([P, QT], bf16, tag="at")
                            nc.scalar.activation(at, ps, Exp, scale=scale)
                            ats.append(at)
                        for e in range(2):
                            for m0 in range(0, QT, MMN):
                                ml = min(MMN, QT - m0)
                                nc.tensor.matmul(
                                    pos[e][:, m0:m0 + ml],
                                    vaug_sb[:, kc, 2 * p + e, :],
                                    ats[e][:, m0:m0 + ml],
                                    start=(kc == 0),
                                    stop=(kc == KC - 1),
                                )
                    for e in range(2):
                        po = pos[e]
                        # one fast copy frees the PSUM slot; the whole
                        # normalize chain then runs off the critical path
                        stg = work.tile([hd + 1, QT], f32, tag="stg")
                        nc.vector.tensor_copy(stg, po)
                        # bounce sums row via DRAM for a stride-0
                        # partition-broadcast read (DMA engines are idle)
                        sums_dr = dscr.tile([1, QT], f32, tag="sums_dr")
                        nc.sync.dma_start(out=sums_dr, in_=stg[hd:hd + 1, :])
                        rec_src = work.tile([hd, QT], f32, tag="rsrc")
                        nc.sync.dma_start(
                            out=rec_src,
                            in_=sums_dr.partition_broadcast(hd))
                        rec = work.tile([hd, QT], f32, tag="rec")
                        nc.vector.reciprocal_approx_fast(rec, rec_src)
                        nc.vector.tensor_mul(
                            aoT_sb[e * hd:(e + 1) * hd, p, q0:q0 + QT],
                            stg[0:hd, :],
                            rec,
                        )
            for tb in range(TB):
                proj_block(tb)

    nc.compile()
    return nc


def get_program(tok=N, d=D, h_loc=HLOC, hd=HD, debug=False, repeat=1):
    key = (tok, d, h_loc, hd, debug, repeat)
    if key not in _PROG_CACHE:
        _PROG_CACHE[key] = _build_program(tok, d, h_loc, hd, debug=debug,
                                          repeat=repeat)
    return _PROG_CACHE[key]


def make_in_maps(inputs_f32, W_qkv, W_proj, b_proj):
    """Shard full inputs into the 8 per-core input dicts."""
    import ml_dtypes

    bf16 = ml_dtypes.bfloat16
    in_maps = []
    for core in range(NCORES):
        b_idx, j = divmod(core, TP)
        f0, f1 = j * FDIM, (j + 1) * FDIM
        xT = np.ascontiguousarray(inputs_f32[b_idx].T).astype(bf16)
        wq_s = np.ascontiguousarray(W_qkv[:, f0:f1]).astype(bf16)
        wk_s = np.ascontiguousarray(W_qkv[:, D + f0:D + f1]).astype(bf16)
        wv_s = np.ascontiguousarray(W_qkv[:, 2 * D + f0:2 * D + f1]).astype(bf16)
        wp_s = np.ascontiguousarray(W_proj[f0:f1, :]).astype(bf16)
        if j == 0:
            bias_rep = np.broadcast_to(b_proj.astype(np.float32), (128, D)).copy()
        else:
            bias_rep = np.zeros((128, D), np.float32)
        in_maps.append(
            {"xT": xT, "wq": wq_s, "wk": wk_s, "wv": wv_s, "wp": wp_s,
             "bias": bias_rep}
        )
    return in_maps


def kernel(inputs, W_qkv, W_proj, b_proj):
    from concourse.bass_utils import run_bass_kernel_spmd

    inputs = np.asarray(inputs, dtype=np.float32)
    W_qkv = np.asarray(W_qkv, dtype=np.float32)
    W_proj = np.asarray(W_proj, dtype=np.float32)
    b_proj = np.asarray(b_proj, dtype=np.float32)

    nc = get_program()
    in_maps = make_in_maps(inputs, W_qkv, W_proj, b_proj)
    res = run_bass_kernel_spmd(nc, in_maps, core_ids=list(range(NCORES)))
    outs = [r["out"].astype(np.float32) for r in res.results]
    full = np.stack([outs[TP * b] + outs[TP * b + 1] for b in range(B)], axis=0)
    return full


# revision 23
# speedup vs baseline: 23.3730x; 1.0644x over previous
"""Multi-head attention (B=4, N=2048, D=1024, H=16) on 8 TRN2 NeuronCores.

Sharding: DP=4 over batch x TP=2 over heads (megatron style).
  core c = 2*batch + j   (j in {0,1} = head-group half)
  - inputs:  x[batch] transposed -> xT [D, N]  (bf16)
  - W_qkv column-sharded: wq/wk/wv = W_qkv[:, {q,k,v} block, heads j*8:(j+1)*8]
  - W_proj row-sharded:   wp = W_proj[j*512:(j+1)*512, :]
  - per-core partial out [N, D]; host sums the TP pair (+ bias via per-core
    bias input that is b_proj on j==0 and zeros on j==1).

Per-core math (bf16 matmuls, fp32 PSUM accumulation):
  qT/kT  [feat, tok] = W^T @ xT          (feature-major, head pairs stacked)
  v      [tok, feat]                     (token-major), augmented with a
                                         ones-column per head (softmax sums)
  per head h, query-tile qh:
    S^T chunk [128 keys, QT] = kT_chunk.T @ qT     (K = hd = 64)
    A^T = exp(scale * S^T)  (ScalarE, PSUM->SBUF, bf16)
    outT_aug [65, QT] += v_aug_chunk.T @ A^T       (row 64 = softmax sums)
    normalize: bcast sums via K=1 matmul, reciprocal_approx_fast, multiply
    -> att_outT [feat, tok] (feature-major == proj lhsT layout)
  proj: out[tok_block] = att_outT_chunk.T @ wp + bias
"""

import numpy as np

B, N, D, H = 4, 2048, 1024, 16
HD = 64
NCORES = 8
TP = 2
HLOC = H // TP          # 8 heads per core
FDIM = HLOC * HD        # 512

_PROG_CACHE = {}


def _build_program(tok, d, h_loc, hd, debug=False, repeat=1):
    """Build the single-core Bass/Tile program (same program runs SPMD on all cores)."""
    import concourse.tile as tile
    from concourse import bacc, mybir

    f32 = mybir.dt.float32
    bf16 = mybir.dt.bfloat16
    Exp = mybir.ActivationFunctionType.Exp

    P = 128
    DC = d // P                 # contraction chunks for QKV (8)
    NP = h_loc // 2             # head pairs (4)
    TB = tok // P               # token blocks (16)
    KC = tok // P               # key chunks (16)
    fdim = h_loc * hd           # local feature dim (512)
    FC = fdim // P              # proj contraction chunks (4)
    QT = min(tok, 1024)         # query tile
    QH = tok // QT              # query-halves (2)
    MMN = 512                   # max matmul free dim per instruction
    scale = float(hd) ** -0.5

    nc = bacc.Bacc("TRN2", target_bir_lowering=False, debug=debug)

    xT = nc.dram_tensor("xT", [d, tok], bf16, kind="ExternalInput")
    wq = nc.dram_tensor("wq", [d, fdim], bf16, kind="ExternalInput")
    wk = nc.dram_tensor("wk", [d, fdim], bf16, kind="ExternalInput")
    wv = nc.dram_tensor("wv", [d, fdim], bf16, kind="ExternalInput")
    wp = nc.dram_tensor("wp", [fdim, d], bf16, kind="ExternalInput")
    bias = nc.dram_tensor("bias", [P, d], f32, kind="ExternalInput")
    out = nc.dram_tensor("out", [tok, d], f32, kind="ExternalOutput")

    with tile.TileContext(nc) as tc:
        with (
            tc.tile_pool(name="sing", bufs=1) as sing,
            tc.tile_pool(name="psA", bufs=2, space="PSUM") as psA,
            tc.tile_pool(name="psO", bufs=2, space="PSUM") as psO,
            tc.tile_pool(name="work", bufs=4) as work,
            tc.tile_pool(name="outp", bufs=3) as outp,
            tc.tile_pool(name="dscr", bufs=4, space="DRAM") as dscr,
        ):
          for _rep in range(repeat):
            # ---- resident loads (xT chunked so compute starts ASAP) -----------
            wv_sb = sing.tile([P, DC, fdim], bf16)
            nc.gpsimd.dma_start(out=wv_sb, in_=wv[:, :].rearrange("(c p) m -> p c m", p=P))
            xT_sb = sing.tile([P, DC, tok], bf16)
            for c in range(DC):
                nc.sync.dma_start(
                    out=xT_sb[:, c, :],
                    in_=xT[c * P:(c + 1) * P, :])
            wq_sb = sing.tile([P, DC, fdim], bf16)
            nc.gpsimd.dma_start(out=wq_sb, in_=wq[:, :].rearrange("(c p) m -> p c m", p=P))
            wk_sb = sing.tile([P, DC, fdim], bf16)
            nc.gpsimd.dma_start(out=wk_sb, in_=wk[:, :].rearrange("(c p) m -> p c m", p=P))
            wp_sb = sing.tile([P, FC, d], bf16)
            nc.gpsimd.dma_start(out=wp_sb, in_=wp[:, :].rearrange("(c p) m -> p c m", p=P))
            bias_sb = sing.tile([P, d], f32)
            nc.gpsimd.dma_start(out=bias_sb, in_=bias[:, :])

            qT_sb = sing.tile([P, NP, tok], bf16)
            kT_sb = sing.tile([P, NP, tok], bf16)
            vaug_sb = sing.tile([P, KC, h_loc, hd + 1], bf16)
            nc.vector.memset(vaug_sb, 1.0)  # ones column survives the v copies
            aoT_sb = sing.tile([P, NP, tok], bf16)

            # ---- v (token-major) into vaug -------------------------------------
            for tb in range(TB):
                ps = psA.tile([P, QT], f32, tag="ps")
                for c in range(DC):
                    for m0 in range(0, fdim, MMN):
                        ml = min(MMN, fdim - m0)
                        nc.tensor.matmul(
                            ps[:, m0:m0 + ml],
                            xT_sb[:, c, tb * P:(tb + 1) * P],
                            wv_sb[:, c, m0:m0 + ml],
                            start=(c == 0),
                            stop=(c == DC - 1),
                        )
                nc.vector.tensor_copy(
                    vaug_sb[:, tb, :, 0:hd],
                    ps[:, 0:fdim].rearrange("p (h e) -> p h e", h=h_loc),
                )

            # ---- q/k projections (all pairs up front) --------------------------
            for p in range(NP):
                for w_sb, dst in ((wq_sb, qT_sb), (wk_sb, kT_sb)):
                    for n0 in range(0, tok, QT):
                        ps = psA.tile([P, QT], f32, tag="ps")
                        for c in range(DC):
                            for m0 in range(0, QT, MMN):
                                ml = min(MMN, QT - m0)
                                nc.tensor.matmul(
                                    ps[:, m0:m0 + ml],
                                    w_sb[:, c, p * P:(p + 1) * P],
                                    xT_sb[:, c, n0 + m0:n0 + m0 + ml],
                                    start=(c == 0),
                                    stop=(c == DC - 1),
                                )
                        nc.vector.tensor_copy(dst[:, p, n0:n0 + QT], ps)

            # ---- attention (qh outer so proj can interleave) -------------------
            def proj_block(tb):
                pp = psA.tile([P, d], f32, tag="ps", name="pp")
                for fc in range(FC):
                    for m0 in range(0, d, MMN):
                        ml = min(MMN, d - m0)
                        nc.tensor.matmul(
                            pp[:, m0:m0 + ml],
                            aoT_sb[:, fc, tb * P:(tb + 1) * P],
                            wp_sb[:, fc, m0:m0 + ml],
                            start=(fc == 0),
                            stop=(fc == FC - 1),
                        )
                ot = outp.tile([P, d], f32, tag="ot", name="ot")
                nc.vector.tensor_add(ot, pp, bias_sb)
                nc.sync.dma_start(out=out[tb * P:(tb + 1) * P, :], in_=ot)

            for qh in range(QH):
                q0 = qh * QT
                for p in range(NP):
                    pos = [psO.tile([hd + 1, QT], f32, tag="po", name=f"po{_e}")
                           for _e in range(2)]
                    for kc in range(KC):
                        ats = []
                        for e in range(2):
                            ps = psA.tile([P, QT], f32, tag="ps")
                            for m0 in range(0, QT, MMN):
                                ml = min(MMN, QT - m0)
                                nc.tensor.matmul(
                                    ps[:, m0:m0 + ml],
                                    kT_sb[e * hd:(e + 1) * hd, p, kc * P:(kc + 1) * P],
                                    qT_sb[e * hd:(e + 1) * hd, p, q0 + m0:q0 + m0 + ml],
                                    start=True,
                                    stop=True,
                                )
                            at = work.tile([P, QT], bf16, tag="at")
                            nc.scalar.activation(at, ps, Exp, scale=scale)
                            ats.append(at)
                        for e in range(2):
                            for m0 in range(0, QT, MMN):
                                ml = min(MMN, QT - m0)
                                nc.tensor.matmul(
                                    pos[e][:, m0:m0 + ml],
                                    vaug_sb[:, kc, 2 * p + e, :],
                                    ats[e][:, m0:m0 + ml],
                                    start=(kc == 0),
                                    stop=(kc == KC - 1),
                                )
                    for e in range(2):
                        po = pos[e]
                        # one fast copy frees the PSUM slot; the whole
                        # normalize chain then runs off the critical path
                        stg = work.tile([hd + 1, QT], f32, tag="stg")
                        nc.vector.tensor_copy(stg, po)
                        # bounce sums row via DRAM for a stride-0
                        # partition-broadcast read (DMA engines are idle)
                        sums_dr = dscr.tile([1, QT], f32, tag="sums_dr")
                        nc.sync.dma_start(out=sums_dr, in_=stg[hd:hd + 1, :])
                        rec_src = work.tile([hd, QT], f32, tag="rsrc")
                        nc.sync.dma_start(
                            out=rec_src,
                            in_=sums_dr.partition_broadcast(hd))
                        rec = work.tile([hd, QT], f32, tag="rec")
                        nc.vector.reciprocal_approx_fast(rec, rec_src)
                        nc.vector.tensor_mul(
                            aoT_sb[e * hd:(e + 1) * hd, p, q0:q0 + QT],
                            stg[0:hd, :],
                            rec,
                        )
            for tb in range(TB):
                proj_block(tb)

    nc.compile()
    return nc


def get_program(tok=N, d=D, h_loc=HLOC, hd=HD, debug=False, repeat=1):
    key = (tok, d, h_loc, hd, debug, repeat)
    if key not in _PROG_CACHE:
        _PROG_CACHE[key] = _build_program(tok, d, h_loc, hd, debug=debug,
                                          repeat=repeat)
    return _PROG_CACHE[key]


def make_in_maps(inputs_f32, W_qkv, W_proj, b_proj):
    """Shard full inputs into the 8 per-core input dicts."""
    import ml_dtypes

    bf16 = ml_dtypes.bfloat16
    in_maps = []
    for core in range(NCORES):
        b_idx, j = divmod(core, TP)
        f0, f1 = j * FDIM, (j + 1) * FDIM
        xT = np.ascontiguousarray(inputs_f32[b_idx].T).astype(bf16)
        wq_s = np.ascontiguousarray(W_qkv[:, f0:f1]).astype(bf16)
        wk_s = np.ascontiguousarray(W_qkv[:, D + f0:D + f1]).astype(bf16)
        wv_s = np.ascontiguousarray(W_qkv[:, 2 * D + f0:2 * D + f1]).astype(bf16)
        wp_s = np.ascontiguousarray(W_proj[f0:f1, :]).astype(bf16)
        if j == 0:
            bias_rep = np.broadcast_to(b_proj.astype(np.float32), (128, D)).copy()
        else:
            bias_rep = np.zeros((128, D), np.float32)
        in_maps.append(
            {"xT": xT, "wq": wq_s, "wk": wk_s, "wv": wv_s, "wp": wp_s,
             "bias": bias_rep}
        )
    return in_maps


def kernel(inputs, W_qkv, W_proj, b_proj):
    from concourse.bass_utils import run_bass_kernel_spmd

    inputs = np.asarray(inputs, dtype=np.float32)
    W_qkv = np.asarray(W_qkv, dtype=np.float32)
    W_proj = np.asarray(W_proj, dtype=np.float32)
    b_proj = np.asarray(b_proj, dtype=np.float32)

    nc = get_program()
    in_maps = make_in_maps(inputs, W_qkv, W_proj, b_proj)
    res = run_bass_kernel_spmd(nc, in_maps, core_ids=list(range(NCORES)))
    outs = [r["out"].astype(np.float32) for r in res.results]
    full = np.stack([outs[TP * b] + outs[TP * b + 1] for b in range(B)], axis=0)
    return full
